# revision 24
# baseline (speedup 1.0000x reference)
"""Trainium2 Bass kernel for nn_AttentionHead_51178830299302.

Single attention head: B=8, S=2048, E=1024, H=64, fp32 I/O, decoder
(causal) masking plus a pad-pad coupling term (padded queries attend
bidirectionally to padded keys).

Strategy:
  * Data-parallel over batch: one batch element per NeuronCore (8 cores).
  * Host-side, each sequence is stably partitioned into [pad | valid]
    positions.  The masked softmax decomposes exactly into two
    independent attention problems: pad x pad (bidirectional, no mask)
    and valid x valid (plain causal), skipping ~60% of the S x S work.
  * The k-projection bias is dropped entirely: softmax_j((q+bq)·(k_j+bk))
    equals softmax_j((q+bq)·k_j) because the difference (q+bq)·bk is
    constant over j.  Scores contract only the 64 head dims (K=64), and
    slot-padded keys are killed through the appended ones-column of V
    (0 at killed rows), which removes them from both the numerator and
    the softmax denominator.
  * DMA is the backbone: the two HWDGE queues deliver only ~150 GB/s
    each here (vNC-shared HBM), so the ~6 MB input stream is what the
    schedule is built around.  Query blocks are slice-aligned and each
    block's scores are emitted immediately after the qk slice that
    covers its queries, keeping the ScalarE exp stream (the serial
    floor, ~17 us) dense from first data to last.  The DMA-completion
    semaphore pool is small and recycled in emission order, so the five
    fast constant loads are placed right before the bulk: they — not
    20 us-late bulk transfers — become the lane predecessors of the
    latency-critical k-bounces and V transposes.
  * Score pairs land in one 2-bank PSUM tile so exp() is batched per
    pair ([P, 2, w] straight from PSUM, bias -3 keeps the pad path in
    fp8 e4m3 range; e^-3 cancels in the host divide).  Pad softmax
    weights and V are fp8 e4m3, AV contracts 2 key-chunks per matmul
    via DoubleRow; the causal part stays bf16.
  * Emission interleaves projections with attention so the PE never
    idles long enough for the HAM activity monitor to drop the clock
    2.4->1.2 GHz, with zero-matmul fillers riding the exp-bound tail.
  * Output is fp16 (host divides in f32); most stores ride the GpSimd
    SWDGE, only the last block's store sits on the sync ring.

kernel(**inputs) takes the FULL unsharded fp32 inputs and returns the
FULL [8, 2048, 64] fp32 output.
"""

import numpy as np
import ml_dtypes

B, S, E, H = 8, 2048, 1024, 64
NEG = -100000.0
P = 128
BF = ml_dtypes.bfloat16
F8 = ml_dtypes.float8_e4m3
F16 = np.float16

_NC_CACHE: dict = {}


def _patch_tile_drain():
    """The stock TileContext exit hangs every global-clock wait on a single
    Drain instruction; this container's walrus caps sync waits at 1 per
    instruction.  Split the waits across single-wait nops, and drop the
    second (post-semclear) all-engine barrier — engines halt right after,
    and NEFF re-execution only starts once every engine has halted."""
    import concourse.tile as tile
    import concourse.mybir as mybir
    from bass_rust import ScopedClock

    if getattr(tile.TileContext, "_drain_waits_split", False):
        return

    def _drain_and_barrier(self, tick_clock, wait_clock):
        nc = self.nc
        carrier = nc.sync.nop(nofuse=True)
        wait_clock.add_sem_waits(
            carrier.ins, ScopedClock({None: tick_clock.global_clock})
        )
        si = carrier.ins.sync_info
        waits = list(si.on_wait) if si and si.on_wait else []
        if len(waits) > 1:
            si.on_wait = waits[:1]
            for w in waits[1:]:
                n = nc.sync.nop(nofuse=True)
                nsi = n.ins.sync_info
                if nsi is None:
                    n.ins.sync_info = mybir.SyncInfo(on_wait=[w], on_update=[])
                else:
                    nsi.on_wait = [w]
        nc.sync.drain()
        nc.all_engine_barrier(sem_only=True)
        popped = nc._tile_sem_poison_stack.pop()
        assert popped is self._sem_poison
        nc.clear_and_free_semaphores(list(self.sems.allocated().values()))

    tile.TileContext._drain_and_barrier = _drain_and_barrier
    tile.TileContext._drain_waits_split = True


def _patch_sync_wait_split():
    """This container's walrus codegen rejects instructions carrying more
    than one sync wait.  Post-process the serialized BIR: hoist excess
    waits onto injected NoOps on the same engine, just before the
    instruction (the sequencer executes them in order, so semantics are
    preserved)."""
    import json
    import concourse.bass as bass

    if getattr(bass.Bass, "_sync_wait_split", False):
        return
    orig = bass.Bass.to_json_bytes

    def to_json_bytes(self) -> bytes:
        j = json.loads(orig(self))
        ctr = [0]

        def fix_block(blk):
            insts = blk.get("instructions")
            if not isinstance(insts, list):
                return
            out = []
            for inst in insts:
                si = inst.get("sync_info")
                ow = (si or {}).get("on_wait") or []
                if len(ow) > 1:
                    si["on_wait"] = ow[-1:]
                    for w in ow[:-1]:
                        ctr[0] += 1
                        out.append(
                            {
                                "debug": inst.get("debug", 0),
                                "engine": inst["engine"],
                                "ins": [],
                                "name": f"I-wsplit-{ctr[0]}",
                                "opcode": "NoOp",
                                "outs": [],
                                "sync_info": {"on_wait": [w], "on_update": []},
                            }
                        )
                out.append(inst)
            blk["instructions"] = out

        def rec(o):
            if isinstance(o, dict):
                if "instructions" in o:
                    fix_block(o)
                for v in o.values():
                    rec(v)
            elif isinstance(o, list):
                for v in o:
                    rec(v)

        rec(j)
        return json.dumps(j).encode()

    bass.Bass.to_json_bytes = to_json_bytes
    bass.Bass._sync_wait_split = True


def build_nc(SV: int, SP: int):
    """Build the SPMD per-core Bass program.

    Per-core DRAM tensors:
      hsT    [P, NSL, 8, 512] bf16  sorted hidden state, transposed
      wqk    [P, 8, 128]      bf16  [Wq/sqrt(H) | Wk]
      wv     [P, 8, H]        bf16
      cst    [P, 129]         bf16  col 0 bq/sqrt(H); cols 1:129 the
                                    tril keep-mask c01[j, y] = (j <= y)
      kill8  [P, NKC_P, 1]    f8    1 at real pad keys, 0 at slot-pads
      killbf [P, NKC_V, 1]    bf16  1 at real valid keys, 0 at slot-pads
      outT   [65, SVP]        f16   rows 0..63 unnormalized output^T,
                                    row 64 softmax denominators
    """
    import concourse.bass as bass
    import concourse.mybir as mybir
    import concourse.tile as tile
    from contextlib import ExitStack

    _patch_tile_drain()
    _patch_sync_wait_split()
    bf, f32, f16 = mybir.dt.bfloat16, mybir.dt.float32, mybir.dt.float16
    f8 = mybir.dt.float8e4
    DR = mybir.MatmulPerfMode.DoubleRow
    Exp = mybir.ActivationFunctionType.Exp

    SVP = SV + SP
    NKC_V, NKC_P = SV // P, SP // P
    NT = SVP // P
    NSL = (SVP + 511) // 512

    nc = bass.Bass("TRN2", target_bir_lowering=False, debug=False)
    hsT_d = nc.dram_tensor("hsT", [P, NSL, 8, 512], bf, kind="ExternalInput").ap()
    wqk_d = nc.dram_tensor("wqk", [P, 8, P], bf, kind="ExternalInput").ap()
    wv_d = nc.dram_tensor("wv", [P, 8, H], bf, kind="ExternalInput").ap()
    cst_d = nc.dram_tensor("cst", [P, 1 + P], bf, kind="ExternalInput").ap()
    kill8_d = nc.dram_tensor("kill8", [P, NKC_P, 1], f8, kind="ExternalInput").ap()
    killbf_d = nc.dram_tensor(
        "killbf", [P, NKC_V, 1], bf, kind="ExternalInput"
    ).ap()
    outT_d = nc.dram_tensor("outT", [H + 1, SVP], f16, kind="ExternalOutput").ap()

    with tile.TileContext(nc) as tc, ExitStack() as ctx:
        singles = ctx.enter_context(tc.tile_pool(name="singles", bufs=1))

        wz = singles.tile([P, 512], bf)
        nc.gpsimd.memset(wz[:], 0.0)

        wqk_s = singles.tile([P, 8, P], bf)
        wv_s = singles.tile([P, 8, H], bf)
        # packed constants: col 0 = bq/sqrt(H) (rows 0:64, bf16), cols
        # 1:129 = the c01 tril keep-mask
        cst_s = singles.tile([P, 1 + P], bf)
        c01_s = cst_s[:, 1 : 1 + P]
        bq_s = singles.tile([H, 1], f32)
        exp_warm = singles.tile([1, 1], bf)
        # exp runs as exp(s - 3) so the pad path stays inside fp8 e4m3
        # range; the factor e^-3 cancels in the host num/den divide.
        nbias = singles.tile([P, 1], f32)
        nc.gpsimd.memset(nbias[:], -3.0)

        # q/k head rows (no augmented rows: slot-pad keys are killed via
        # the V kill column, keeping score K=64).  k projects to PSUM
        # rows 64:128 and evacuates in place to kT_hi; a DMA bounce
        # brings it down to kT rows 0:64 where the score matmuls (whose
        # operands must share a partition range) read it.
        qT = singles.tile([H, SVP], bf)
        kT = singles.tile([H, SVP], bf)
        kT_hi = singles.tile([P, SVP], bf)

        # V in natural [key-part, head] layout; column H holds the kill
        # 0/1 flags and rides the AV matmul as the softmax denominator
        # row.  fp8 for pad chunks (DoubleRow AV; free dim padded 65->80
        # for the 16B DoubleRow chunk stride), bf16 for valid chunks.
        vS8 = singles.tile([P, NKC_P, 80], f8)
        vS_bf = singles.tile([P, NKC_V, H + 1], bf)
        vT = singles.tile([H, SVP], bf)
        # XBAR transpose needs a contiguous destination; stage here, then
        # strided-copy into vS (which carries the kill column).
        vN = singles.tile([P, NT, H], bf)
        # kill flags come in contiguous (tiny strided DMAs are glacial
        # on the SWDGE) and are copied into the vS kill columns on-chip
        k8st = singles.tile([P, NKC_P, 1], f8)
        kbfst = singles.tile([P, NKC_V, 1], bf)

        hsT = singles.tile([P, NSL, 8, 512], bf)

        # ------- DMA plan -------
        # Emission order doubles as the DMA-sem lane order; constants sit
        # right before the bulk so later latency-critical transfers have
        # fast lane predecessors.  Scalar(Q10) carries the h0 halves plus
        # the whole tail slice and then goes quiet for exp; sync(Q1)
        # carries wqk + early h1 halves, then serves bounces/transposes
        # just-in-time (s2h1/s3h1 are emitted mid-schedule so they don't
        # block those in the ring FIFO).
        def load_half(si, h, eng):
            w2 = min(512, SVP - si * 512)
            eng.dma_start(
                hsT[:, si, 4 * h : 4 * h + 4, :w2],
                hsT_d[:, si, 4 * h : 4 * h + 4, :w2],
            )

        def load_full(si, eng):
            w2 = min(512, SVP - si * 512)
            eng.dma_start(hsT[:, si, :, :w2], hsT_d[:, si, :, :w2])

        # The DMA-completion sem pool holds ~13 entries, recycled in
        # scheduled order; a trigger whose recycled sem isn't free yet
        # blocks the whole engine queue behind it (on scalar that queue
        # carries the exp stream).  Keep the pre-compute transfer count
        # AT the pool size so no pre-compute trigger ever waits, and all
        # mid-schedule latency-critical transfers recycle the sems of
        # fast early transfers.
        nc.scalar.dma_start(wqk_s[:], wqk_d)
        load_half(0, 1, nc.sync)
        load_half(0, 0, nc.scalar)
        load_half(1, 0, nc.scalar)
        load_half(1, 1, nc.sync)
        load_half(2, 0, nc.scalar)
        load_half(2, 1, nc.sync)
        for si in range(3, NSL):
            load_full(si, nc.scalar)
        nc.gpsimd.dma_start(wv_s[:], wv_d)
        nc.gpsimd.dma_start(cst_s[:], cst_d)
        nc.gpsimd.dma_start(k8st[:], kill8_d)
        nc.gpsimd.dma_start(kbfst[:], killbf_d)
        nc.gpsimd.tensor_copy(vS8[:, :, H : H + 1], k8st[:])
        nc.gpsimd.tensor_copy(vS_bf[:, :, H : H + 1], kbfst[:])
        nc.gpsimd.tensor_copy(bq_s[:], cst_s[0:H, 0:1])  # bf16 -> f32

        with tc.tile_pool(name="pp", bufs=2, space="PSUM") as pp, \
             tc.tile_pool(name="acc", bufs=2, space="PSUM") as acc, \
             tc.tile_pool(name="spsum", bufs=2, space="PSUM") as spsum, \
             tc.tile_pool(name="wpool", bufs=8) as wpool, \
             tc.tile_pool(name="opool", bufs=2) as opool:

            # dummy exp pulls the ~2.7us ACT_TABLE_LOAD out of the
            # attention pipeline (right after scalar's DMA triggers).
            nc.scalar.activation(exp_warm[:], wz[0:1, 0:1], Exp)
            # PE warm-up: fills the time until hsT slice 0 lands and
            # ramps the HAM p-state (1.2 -> 2.4 GHz after ~3.4us busy).
            warm_ps = pp.tile([P, 512], f32, tag="ps", name="warm_ps")
            for _ in range(20):
                nc.tensor.matmul(
                    warm_ps[:, 0:256], lhsT=wz[:, 0:P], rhs=wz[:, 0:256],
                    start=True, stop=True,
                )

            def emit_filler(n):
                # keep-warm zero matmuls: execute during exp waits in the
                # attention tail so the HAM activity window stays busy.
                fps = pp.tile([P, 512], f32, tag="ps", name="fps")
                for _ in range(n):
                    nc.tensor.matmul(
                        fps[:, 0:256], lhsT=wz[:, 0:P], rhs=wz[:, 0:256],
                        start=True, stop=True, skip_group_check=True,
                    )

            def emit_qk_slice(sb):
                si = sb // 512
                w = min(512, SVP - sb)
                ps = pp.tile([P, 512], f32, tag="ps", name="ps")
                for c in range(8):
                    nc.tensor.matmul(
                        ps[:, :w],
                        lhsT=wqk_s[:, c, :],
                        rhs=hsT[:, si, c, :w],
                        start=(c == 0),
                        stop=(c == 7),
                    )
                # q on partitions 0:64 -> evacuate (+bq) straight into qT
                nc.vector.tensor_scalar_add(
                    qT[:, sb : sb + w], ps[0:64, :w], bq_s[:, 0:1]
                )
                # k on partitions 64:128 -> evacuate in place, bounce to
                # rows 0:64.  No k bias: it cancels in the softmax.
                nc.vector.tensor_copy(
                    kT_hi[64:128, sb : sb + w], ps[64:128, :w]
                )
                # SBUF->SBUF bounce rides the SWDGE: the sync ring is
                # busy with bulk h1 halves exactly when early bounces
                # must land, and SWDGE doesn't touch HBM bandwidth.
                nc.gpsimd.dma_start(
                    kT[:, sb : sb + w], kT_hi[64:128, sb : sb + w]
                )

            def copy_vS(ta, tb):
                # route chunks to the part-specific V tile on GpSimd
                # (idle here; keeps vector out of the AV dependency chain)
                if ta < NKC_P:
                    e = min(tb, NKC_P)
                    nc.gpsimd.tensor_copy(vS8[:, ta:e, 0:H], vN[:, ta:e, :])
                if tb > NKC_P:
                    b = max(ta, NKC_P)
                    nc.gpsimd.tensor_copy(
                        vS_bf[:, b - NKC_P : tb - NKC_P, 0:H], vN[:, b:tb, :]
                    )

            def emit_v_slice(sb):
                si = sb // 512
                w = min(512, SVP - sb)
                pvd = pp.tile([H, 512], f32, tag="ps", name="pvd")
                for c in range(8):
                    nc.tensor.matmul(
                        pvd[:, :w],
                        lhsT=wv_s[:, c, :],
                        rhs=hsT[:, si, c, :w],
                        start=(c == 0),
                        stop=(c == 7),
                    )
                nc.vector.tensor_copy(vT[:, sb : sb + w], pvd[:, :w])

            def emit_v_transpose(sa, sb_end):
                # one XBAR transpose covering [sa, sb_end) columns
                ta, tb = sa // P, sb_end // P
                nc.sync.dma_start_transpose(vN[:, ta:tb, :], vT[:, sa:sb_end])
                copy_vS(ta, tb)

            class QBlock:
                """One slice-aligned query block.  part 0 = pad
                (bidirectional keys 0..SP, fp8 DoubleRow AV), part 1 =
                valid (causal, bf16 AV).  Score pairs go into a 2-bank
                PSUM tile; exp is batched over the pair."""

                def __init__(self, part, q0r, w):
                    self.part = part
                    self.causal = part == 1
                    part_q0 = 0 if part == 0 else SP
                    self.kc_base = 0 if part == 0 else NKC_P
                    self.w = w
                    self.q0r = q0r
                    self.q0 = part_q0 + q0r
                    if self.causal:
                        self.kcs = list(range(0, (q0r + w - 1) // P + 1))
                    else:
                        self.kcs = list(range(NKC_P))
                    self.n_kc = len(self.kcs)
                    self.ki = 0
                    self.pend = []  # (kcr, wt_tile, j, y0, d)
                    self.ot = None
                    self.st = None
                    self.wt = None
                    self.slot = 0  # pair-slot within current st/wt tile
                    self.slots = 512 // self.w

                def _d(self, kcr):
                    return max(kcr * P - self.q0r, 0) if self.causal else 0

                def scores(self, kcrs, fillers=0):
                    if not kcrs:
                        return
                    if self.ot is None:
                        self.ot = acc.tile(
                            [H + 1, 512], f32, tag="acc", name="ot"
                        )
                    w, q0 = self.w, self.q0
                    i = 0
                    while i < len(kcrs):
                        pair = kcrs[i : i + 2]
                        i += len(pair)
                        if self.slot == 0:
                            self.st = spsum.tile(
                                [P, 2, 512], f32, tag="st", name="st"
                            )
                            if self.causal:
                                self.wt = wpool.tile(
                                    [P, 2, 512], bf, tag="wtb", name="wtb"
                                )
                            else:
                                self.wt = wpool.tile(
                                    [P, 2, 512], f8, tag="wt", name="wt"
                                )
                        st, wt, y0 = self.st, self.wt, self.slot * self.w
                        ds = [self._d(k) for k in pair]
                        for j, (kcr, d) in enumerate(zip(pair, ds)):
                            kc = self.kc_base + kcr
                            nc.tensor.matmul(
                                st[:, j, y0 + d : y0 + w],
                                lhsT=kT[:, kc * P : (kc + 1) * P],
                                rhs=qT[:, q0 + d : q0 + w],
                                start=True,
                                stop=True,
                                skip_group_check=True,
                            )
                        if fillers:
                            emit_filler(fillers)
                        if len(pair) == 2 and ds[0] == ds[1]:
                            nc.scalar.activation(
                                wt[:, 0:2, y0 + ds[0] : y0 + w],
                                st[:, 0:2, y0 + ds[0] : y0 + w],
                                Exp, bias=nbias[:, 0:1],
                            )
                        else:
                            for j, (kcr, d) in enumerate(zip(pair, ds)):
                                nc.scalar.activation(
                                    wt[:, j, y0 + d : y0 + w],
                                    st[:, j, y0 + d : y0 + w],
                                    Exp, bias=nbias[:, 0:1],
                                )
                        if self.causal:
                            for j, (kcr, d) in enumerate(zip(pair, ds)):
                                if kcr * P - self.q0r >= 0:
                                    # diagonal block: only the 128-wide
                                    # strip at d needs the intra-block
                                    # tril; later cols are all-keep
                                    de = min(d + P, w)
                                    nc.vector.tensor_mul(
                                        wt[:, j, y0 + d : y0 + de],
                                        wt[:, j, y0 + d : y0 + de],
                                        c01_s[:, 0 : de - d],
                                    )
                        for j, (kcr, d) in enumerate(zip(pair, ds)):
                            self.pend.append((kcr, wt, j, y0, d))
                        self.slot = (self.slot + 1) % self.slots

                def avs(self, n_chunks=None):
                    # consume pend'd exp'd chunks: fp8 DoubleRow pairs for
                    # pad, bf16 singles for causal
                    w, left = self.w, n_chunks
                    while self.pend and (left is None or left > 0):
                        if (
                            not self.causal
                            and len(self.pend) >= 2
                            and self.pend[0][1] is self.pend[1][1]
                            and self.pend[0][2] == 0
                            and self.pend[0][3] == self.pend[1][3]
                        ):
                            (kc0, wt, _, y0, _) = self.pend[0]
                            kc1 = self.pend[1][0]
                            assert kc1 == kc0 + 1
                            nc.tensor.matmul(
                                self.ot[:, :w],
                                lhsT=vS8[:, kc0 : kc0 + 2, 0 : H + 1],
                                rhs=wt[:, 0:2, y0 : y0 + w],
                                start=(self.ki == 0),
                                stop=(self.ki + 2 == self.n_kc),
                                perf_mode=DR,
                                skip_group_check=True,
                            )
                            self.ki += 2
                            del self.pend[:2]
                            if left is not None:
                                left -= 2
                        else:
                            kcr, wt, j, y0, d = self.pend[0]
                            vsrc = (
                                vS_bf[:, kcr, :]
                                if self.causal
                                else vS8[:, kcr, 0 : H + 1]
                            )
                            nc.tensor.matmul(
                                self.ot[:, d:w],
                                lhsT=vsrc,
                                rhs=wt[:, j, y0 + d : y0 + w],
                                start=(self.ki == 0),
                                stop=(self.ki + 1 == self.n_kc),
                                skip_group_check=True,
                            )
                            self.ki += 1
                            del self.pend[:1]
                            if left is not None:
                                left -= 1

                def finish(self, store_engine):
                    self.avs()
                    assert self.ki == self.n_kc
                    w = self.w
                    osb = opool.tile([H + 1, 512], f16)
                    nc.vector.tensor_copy(osb[:, :w], self.ot[:, :w])
                    store_engine.dma_start(
                        outT_d[:, self.q0 : self.q0 + w], osb[:, :w]
                    )

            # ------- master schedule -------
            # Pad blocks are 512-aligned within [0, SP); causal blocks
            # are aligned to the ABSOLUTE 512-grid within [SP, SVP) so
            # each causal block's queries live in exactly one hsT slice
            # and its scores can fire right after that slice's qk.
            slice_starts = list(range(0, SVP, 512))
            pad_blocks = [
                QBlock(0, r, min(512, SP - r)) for r in range(0, SP, 512)
            ]
            cuts = [SP] + [
                c for c in range(((SP // 512) + 1) * 512, SVP, 512)
            ] + [SVP]
            causal_blocks = [
                QBlock(1, a - SP, b - a) for a, b in zip(cuts, cuts[1:])
            ]
            # causal block -> index of the qk slice covering its queries
            c_by_slice = {}
            for blk in causal_blocks:
                c_by_slice.setdefault((SP + blk.q0r) // 512, []).append(blk)

            kc_s01 = min(NKC_P, 1024 // P)
            P0 = pad_blocks[0]
            P1 = pad_blocks[1] if len(pad_blocks) > 1 else None

            emit_qk_slice(slice_starts[0])
            emit_qk_slice(slice_starts[1])
            P0.scores(P0.kcs[:kc_s01])
            if P1 is not None:
                P1.scores(P1.kcs[:kc_s01])

            emit_v_slice(slice_starts[0])
            emit_v_slice(slice_starts[1])
            emit_v_transpose(0, min(1024, SVP))

            if NSL > 2:
                emit_qk_slice(slice_starts[2])
                # v2 fills the PE while kb2 (SWDGE) is in flight, and
                # its transpose covers pad chunk 8 for the pad AV8s
                emit_v_slice(slice_starts[2])
                emit_v_transpose(1024, min(1536, SVP))
            P0.scores(P0.kcs[kc_s01:])
            for blk in pad_blocks[2:]:
                blk.scores(blk.kcs)
            for blk in c_by_slice.get(2, []):
                blk.scores(blk.kcs, fillers=1)
            if NSL > 3:
                emit_qk_slice(slice_starts[3])
            P0.avs(2 * (kc_s01 // 2))
            if P1 is not None:
                P1.avs(2 * (kc_s01 // 2))
                P1.scores(P1.kcs[kc_s01:])
            for blk in c_by_slice.get(3, []):
                blk.scores(blk.kcs, fillers=1)

            for si in range(4, NSL):
                emit_qk_slice(slice_starts[si])
                for blk in c_by_slice.get(si, []):
                    blk.scores(blk.kcs, fillers=1)

            for sb in slice_starts[3:]:
                emit_v_slice(sb)
                emit_v_transpose(sb, min(sb + 512, SVP))

            # finishes in ot-pool rotation order (= scores order)
            fin_order = []
            seen = set()
            for blk in (
                pad_blocks[:2]
                + c_by_slice.get(2, [])
                + pad_blocks[2:]
                + [b for bs in sorted(c_by_slice) for b in c_by_slice[bs]]
            ):
                if id(blk) not in seen:
                    seen.add(id(blk))
                    fin_order.append(blk)
            for i, blk in enumerate(fin_order):
                last = i == len(fin_order) - 1
                blk.finish(nc.sync if last else nc.gpsimd)
    return nc


def _prepare(hidden_state, attention_masks, Wq, bq, Wk, bk, Wv, bv):
    """Host-side shard prep: sort each sequence into [pad | valid],
    pad both groups to shared multiples of 128, cast to bf16."""
    hs = np.asarray(hidden_state, dtype=np.float32)
    m = np.asarray(attention_masks)
    perms, nvs = [], []
    for b in range(B):
        mb = np.asarray(m[b]).astype(np.int64)
        perms.append(np.argsort(1 - mb, kind="stable"))
        nvs.append(int(mb.sum()))
    nps = [S - nv for nv in nvs]
    SV = max(128, -(-max(nvs) // P) * P)
    SPn = max(128, -(-max(nps) // P) * P)
    SVP = SV + SPn
    NKC_P, NKC_V = SPn // P, SV // P

    wqk = np.ascontiguousarray(
        np.concatenate(
            [np.asarray(Wq, np.float32) / np.sqrt(H), np.asarray(Wk, np.float32)],
            axis=1,
        ).reshape(8, P, P).transpose(1, 0, 2)
    ).astype(BF)  # [p, c, m]
    wv = np.ascontiguousarray(
        np.asarray(Wv, np.float32).reshape(8, P, H).transpose(1, 0, 2)
    ).astype(BF)  # [p, c, m]
    # packed constants: col 0 = bq/sqrt(H) (rows 0:64); cols 1:129 the
    # tril keep-mask c01[j, y] = 1.0 iff j <= y
    cst = np.zeros((P, 1 + P), np.float32)
    cst[:H, 0] = np.asarray(bq, np.float32) / np.sqrt(H)
    y = np.arange(P)
    cst[:, 1:] = np.arange(P)[:, None] <= y[None, :]
    cst = cst.astype(BF)

    in_maps = []
    for b in range(B):
        nv, npd = nvs[b], nps[b]
        NSL = (SVP + 511) // 512
        # pad-first layout: pad rows at cols [0, npd), valid rows at
        # [SPn, SPn + nv); slot-padding rows are zero and their keys are
        # killed through the V kill column
        hs_sorted = np.zeros((NSL * 512, E), np.float32)
        hs_sorted[:npd] = hs[b][perms[b][nv:]]
        hs_sorted[SPn : SPn + nv] = hs[b][perms[b][:nv]]
        hsT = np.ascontiguousarray(
            hs_sorted.reshape(NSL, 512, 8, P).transpose(3, 0, 2, 1)
        ).astype(BF)
        kill = np.ones((SVP,), np.float32)
        kill[npd:SPn] = 0.0
        kill[SPn + nv :] = 0.0
        # [P, NKC, 1] layout: kill[kc*128 + p] -> [p, kc, 0]
        killp = kill[:SPn].reshape(NKC_P, P).T[:, :, None]
        killv = kill[SPn:].reshape(NKC_V, P).T[:, :, None]
        in_maps.append(
            {
                "hsT": hsT,
                "wqk": wqk,
                "wv": wv,
                "cst": cst,
                "kill8": np.ascontiguousarray(killp).astype(F8),
                "killbf": np.ascontiguousarray(killv).astype(BF),
            }
        )
    return in_maps, perms, nvs, SV, SPn


def _run(inputs: dict, trace: bool = False):
    from concourse import bass_utils

    in_maps, perms, nvs, SV, SPn = _prepare(**inputs)
    key = (SV, SPn)
    if key not in _NC_CACHE:
        _NC_CACHE[key] = build_nc(SV, SPn)
    nc = _NC_CACHE[key]

    res = bass_utils.run_bass_kernel_spmd(
        nc, in_maps, core_ids=list(range(8)), trace=trace
    )

    bv = np.asarray(inputs["bv"], np.float32)
    out = np.empty((B, S, H), np.float32)
    for b in range(B):
        ot = np.asarray(res.results[b]["outT"], np.float32)  # [65, SVP]
        with np.errstate(divide="ignore", invalid="ignore", over="ignore"):
            dev = (ot[:H] / ot[H]).T  # normalized; slot-pad rows discarded
        nv = nvs[b]
        out[b][perms[b][nv:]] = dev[: S - nv]
        out[b][perms[b][:nv]] = dev[SPn : SPn + nv]
    out += bv  # v-projection bias commutes with the softmax average
    return out, res


def kernel(**inputs) -> np.ndarray:
    out, _ = _run(inputs, trace=False)
    return out


# revision 25
# speedup vs baseline: 1.1250x; 1.1250x over previous
"""Trainium2 Bass kernel for nn_AttentionHead_51178830299302.

Single attention head: B=8, S=2048, E=1024, H=64, fp32 I/O, decoder
(causal) masking plus a pad-pad coupling term (padded queries attend
bidirectionally to padded keys).

Strategy:
  * Data-parallel over batch: one batch element per NeuronCore (8 cores).
  * Host-side, each sequence is stably partitioned into [pad | valid]
    positions.  The masked softmax decomposes exactly into two
    independent attention problems: pad x pad (bidirectional, no mask)
    and valid x valid (plain causal), skipping ~60% of the S x S work.
  * The k-projection bias is dropped entirely: softmax_j((q+bq)·(k_j+bk))
    equals softmax_j((q+bq)·k_j) because the difference (q+bq)·bk is
    constant over j.  Scores contract only the 64 head dims (K=64), and
    slot-padded keys are killed through the appended ones-column of V
    (0 at killed rows), which removes them from both the numerator and
    the softmax denominator.
  * DMA is the backbone: the two HWDGE queues deliver only ~150 GB/s
    each here (vNC-shared HBM), so the ~6 MB input stream is what the
    schedule is built around.  Query blocks are slice-aligned and each
    block's scores are emitted immediately after the qk slice that
    covers its queries, keeping the ScalarE exp stream (the serial
    floor, ~17 us) dense from first data to last.  The DMA-completion
    semaphore pool is small and recycled in emission order, so the five
    fast constant loads are placed right before the bulk: they — not
    20 us-late bulk transfers — become the lane predecessors of the
    latency-critical k-bounces and V transposes.
  * Score pairs land in one 2-bank PSUM tile so exp() is batched per
    pair ([P, 2, w] straight from PSUM, bias -3 keeps the pad path in
    fp8 e4m3 range; e^-3 cancels in the host divide).  Pad softmax
    weights and V are fp8 e4m3, AV contracts 2 key-chunks per matmul
    via DoubleRow; the causal part stays bf16.
  * Emission interleaves projections with attention so the PE never
    idles long enough for the HAM activity monitor to drop the clock
    2.4->1.2 GHz, with zero-matmul fillers riding the exp-bound tail.
  * Output is fp16 (host divides in f32); most stores ride the GpSimd
    SWDGE, only the last block's store sits on the sync ring.

kernel(**inputs) takes the FULL unsharded fp32 inputs and returns the
FULL [8, 2048, 64] fp32 output.
"""

import numpy as np
import ml_dtypes

B, S, E, H = 8, 2048, 1024, 64
NEG = -100000.0
P = 128
BF = ml_dtypes.bfloat16
F8 = ml_dtypes.float8_e4m3
F16 = np.float16

_NC_CACHE: dict = {}


def _patch_tile_drain():
    """The stock TileContext exit hangs every global-clock wait on a single
    Drain instruction; this container's walrus caps sync waits at 1 per
    instruction.  Split the waits across single-wait nops, and drop the
    second (post-semclear) all-engine barrier — engines halt right after,
    and NEFF re-execution only starts once every engine has halted."""
    import concourse.tile as tile
    import concourse.mybir as mybir
    from bass_rust import ScopedClock

    if getattr(tile.TileContext, "_drain_waits_split", False):
        return

    def _drain_and_barrier(self, tick_clock, wait_clock):
        nc = self.nc
        carrier = nc.sync.nop(nofuse=True)
        wait_clock.add_sem_waits(
            carrier.ins, ScopedClock({None: tick_clock.global_clock})
        )
        si = carrier.ins.sync_info
        waits = list(si.on_wait) if si and si.on_wait else []
        if len(waits) > 1:
            si.on_wait = waits[:1]
            for w in waits[1:]:
                n = nc.sync.nop(nofuse=True)
                nsi = n.ins.sync_info
                if nsi is None:
                    n.ins.sync_info = mybir.SyncInfo(on_wait=[w], on_update=[])
                else:
                    nsi.on_wait = [w]
        nc.sync.drain()
        nc.all_engine_barrier(sem_only=True)
        popped = nc._tile_sem_poison_stack.pop()
        assert popped is self._sem_poison
        nc.clear_and_free_semaphores(list(self.sems.allocated().values()))

    tile.TileContext._drain_and_barrier = _drain_and_barrier
    tile.TileContext._drain_waits_split = True


def _patch_sync_wait_split():
    """This container's walrus codegen rejects instructions carrying more
    than one sync wait.  Post-process the serialized BIR: hoist excess
    waits onto injected NoOps on the same engine, just before the
    instruction (the sequencer executes them in order, so semantics are
    preserved)."""
    import json
    import concourse.bass as bass

    if getattr(bass.Bass, "_sync_wait_split", False):
        return
    orig = bass.Bass.to_json_bytes

    def to_json_bytes(self) -> bytes:
        j = json.loads(orig(self))
        ctr = [0]

        def fix_block(blk):
            insts = blk.get("instructions")
            if not isinstance(insts, list):
                return
            out = []
            for inst in insts:
                si = inst.get("sync_info")
                ow = (si or {}).get("on_wait") or []
                if len(ow) > 1:
                    si["on_wait"] = ow[-1:]
                    for w in ow[:-1]:
                        ctr[0] += 1
                        out.append(
                            {
                                "debug": inst.get("debug", 0),
                                "engine": inst["engine"],
                                "ins": [],
                                "name": f"I-wsplit-{ctr[0]}",
                                "opcode": "NoOp",
                                "outs": [],
                                "sync_info": {"on_wait": [w], "on_update": []},
                            }
                        )
                out.append(inst)
            blk["instructions"] = out

        def rec(o):
            if isinstance(o, dict):
                if "instructions" in o:
                    fix_block(o)
                for v in o.values():
                    rec(v)
            elif isinstance(o, list):
                for v in o:
                    rec(v)

        rec(j)
        return json.dumps(j).encode()

    bass.Bass.to_json_bytes = to_json_bytes
    bass.Bass._sync_wait_split = True


def build_nc(SV: int, SP: int):
    """Build the SPMD per-core Bass program.

    Per-core DRAM tensors:
      hsT    [P, NSL, 8, 512] bf16  sorted hidden state, transposed
      wqk    [P, 8, 128]      bf16  [Wq/sqrt(H) | Wk]
      wv     [P, 8, H]        bf16
      cst    [P, 129]         bf16  col 0 bq/sqrt(H); cols 1:129 the
                                    tril keep-mask c01[j, y] = (j <= y)
      kill8  [P, NKC_P, 1]    f8    1 at real pad keys, 0 at slot-pads
      killbf [P, NKC_V, 1]    bf16  1 at real valid keys, 0 at slot-pads
      outT   [65, SVP]        f16   rows 0..63 unnormalized output^T,
                                    row 64 softmax denominators
    """
    import concourse.bass as bass
    import concourse.mybir as mybir
    import concourse.tile as tile
    from contextlib import ExitStack

    _patch_tile_drain()
    _patch_sync_wait_split()
    bf, f32, f16 = mybir.dt.bfloat16, mybir.dt.float32, mybir.dt.float16
    f8 = mybir.dt.float8e4
    DR = mybir.MatmulPerfMode.DoubleRow
    Exp = mybir.ActivationFunctionType.Exp

    SVP = SV + SP
    NKC_V, NKC_P = SV // P, SP // P
    NT = SVP // P
    NSL = (SVP + 511) // 512

    nc = bass.Bass("TRN2", target_bir_lowering=False, debug=False)
    hsT_d = nc.dram_tensor("hsT", [P, NSL, 8, 512], bf, kind="ExternalInput").ap()
    wqk_d = nc.dram_tensor("wqk", [P, 8, P], bf, kind="ExternalInput").ap()
    wv_d = nc.dram_tensor("wv", [P, 8, H], bf, kind="ExternalInput").ap()
    cst_d = nc.dram_tensor("cst", [P, 1 + P], bf, kind="ExternalInput").ap()
    kill8_d = nc.dram_tensor("kill8", [P, NKC_P, 1], f8, kind="ExternalInput").ap()
    killbf_d = nc.dram_tensor(
        "killbf", [P, NKC_V, 1], bf, kind="ExternalInput"
    ).ap()
    outT_d = nc.dram_tensor("outT", [H + 1, SVP], f16, kind="ExternalOutput").ap()

    with tile.TileContext(nc) as tc, ExitStack() as ctx:
        singles = ctx.enter_context(tc.tile_pool(name="singles", bufs=1))

        wz = singles.tile([P, 512], bf)
        nc.gpsimd.memset(wz[:], 0.0)

        wqk_s = singles.tile([P, 8, P], bf)
        wv_s = singles.tile([P, 8, H], bf)
        # packed constants: col 0 = bq/sqrt(H) (rows 0:64, bf16), cols
        # 1:129 = the c01 tril keep-mask
        cst_s = singles.tile([P, 1 + P], bf)
        c01_s = cst_s[:, 1 : 1 + P]
        bq_s = singles.tile([H, 1], f32)
        exp_warm = singles.tile([1, 1], bf)
        # exp runs as exp(s - 3) so the pad path stays inside fp8 e4m3
        # range; the factor e^-3 cancels in the host num/den divide.
        nbias = singles.tile([P, 1], f32)
        nc.gpsimd.memset(nbias[:], -3.0)

        # q/k head rows (no augmented rows: slot-pad keys are killed via
        # the V kill column, keeping score K=64).  k projects to PSUM
        # rows 64:128 and evacuates in place to kT_hi; a DMA bounce
        # brings it down to kT rows 0:64 where the score matmuls (whose
        # operands must share a partition range) read it.
        qT = singles.tile([H, SVP], bf)
        kT = singles.tile([H, SVP], bf)
        kT_hi = singles.tile([P, SVP], bf)

        # V in natural [key-part, head] layout; column H holds the kill
        # 0/1 flags and rides the AV matmul as the softmax denominator
        # row.  fp8 for pad chunks (DoubleRow AV; free dim padded 65->80
        # for the 16B DoubleRow chunk stride), bf16 for valid chunks.
        vS8 = singles.tile([P, NKC_P, 80], f8)
        vS_bf = singles.tile([P, NKC_V, H + 1], bf)
        vT = singles.tile([H, SVP], bf)
        # XBAR transpose needs a contiguous destination; stage here, then
        # strided-copy into vS (which carries the kill column).
        vN = singles.tile([P, NT, H], bf)
        # kill flags come in contiguous (tiny strided DMAs are glacial
        # on the SWDGE) and are copied into the vS kill columns on-chip
        k8st = singles.tile([P, NKC_P, 1], f8)
        kbfst = singles.tile([P, NKC_V, 1], bf)

        hsT = singles.tile([P, NSL, 8, 512], bf)

        # ------- DMA plan -------
        # Emission order doubles as the DMA-sem lane order; constants sit
        # right before the bulk so later latency-critical transfers have
        # fast lane predecessors.  Scalar(Q10) carries the h0 halves plus
        # the whole tail slice and then goes quiet for exp; sync(Q1)
        # carries wqk + early h1 halves, then serves bounces/transposes
        # just-in-time (s2h1/s3h1 are emitted mid-schedule so they don't
        # block those in the ring FIFO).
        def load_half(si, h, eng):
            w2 = min(512, SVP - si * 512)
            eng.dma_start(
                hsT[:, si, 4 * h : 4 * h + 4, :w2],
                hsT_d[:, si, 4 * h : 4 * h + 4, :w2],
            )

        def load_full(si, eng):
            w2 = min(512, SVP - si * 512)
            eng.dma_start(hsT[:, si, :, :w2], hsT_d[:, si, :, :w2])

        # The DMA-completion sem pool holds ~13 entries, recycled in
        # scheduled order; a trigger whose recycled sem isn't free yet
        # blocks the whole engine queue behind it (on scalar that queue
        # carries the exp stream).  Keep the pre-compute transfer count
        # AT the pool size so no pre-compute trigger ever waits, and all
        # mid-schedule latency-critical transfers recycle the sems of
        # fast early transfers.
        nc.scalar.dma_start(wqk_s[:], wqk_d)
        load_half(0, 1, nc.sync)
        load_half(0, 0, nc.scalar)
        load_half(1, 0, nc.scalar)
        load_half(1, 1, nc.sync)
        load_half(2, 0, nc.scalar)
        load_half(2, 1, nc.sync)
        for si in range(3, NSL):
            load_full(si, nc.scalar)
        nc.gpsimd.dma_start(wv_s[:], wv_d)
        nc.gpsimd.dma_start(cst_s[:], cst_d)
        nc.gpsimd.dma_start(k8st[:], kill8_d)
        nc.gpsimd.dma_start(kbfst[:], killbf_d)
        nc.gpsimd.tensor_copy(vS8[:, :, H : H + 1], k8st[:])
        nc.gpsimd.tensor_copy(vS_bf[:, :, H : H + 1], kbfst[:])
        nc.gpsimd.tensor_copy(bq_s[:], cst_s[0:H, 0:1])  # bf16 -> f32

        with tc.tile_pool(name="pp", bufs=2, space="PSUM") as pp, \
             tc.tile_pool(name="acc", bufs=2, space="PSUM") as acc, \
             tc.tile_pool(name="spsum", bufs=2, space="PSUM") as spsum, \
             tc.tile_pool(name="wpool", bufs=8) as wpool, \
             tc.tile_pool(name="opool", bufs=2) as opool:

            # dummy exp pulls the ~2.7us ACT_TABLE_LOAD out of the
            # attention pipeline (right after scalar's DMA triggers).
            nc.scalar.activation(exp_warm[:], wz[0:1, 0:1], Exp)
            # PE warm-up: fills the time until hsT slice 0 lands and
            # ramps the HAM p-state (1.2 -> 2.4 GHz after ~3.4us busy).
            warm_ps = pp.tile([P, 512], f32, tag="ps", name="warm_ps")
            for _ in range(14):
                nc.tensor.matmul(
                    warm_ps[:, 0:256], lhsT=wz[:, 0:P], rhs=wz[:, 0:256],
                    start=True, stop=True,
                )

            def emit_filler(n):
                # keep-warm zero matmuls: execute during exp waits in the
                # attention tail so the HAM activity window stays busy.
                fps = pp.tile([P, 512], f32, tag="ps", name="fps")
                for _ in range(n):
                    nc.tensor.matmul(
                        fps[:, 0:256], lhsT=wz[:, 0:P], rhs=wz[:, 0:256],
                        start=True, stop=True, skip_group_check=True,
                    )

            def emit_qk_slice(sb):
                si = sb // 512
                w = min(512, SVP - sb)
                ps = pp.tile([P, 512], f32, tag="ps", name="ps")
                for c in range(8):
                    nc.tensor.matmul(
                        ps[:, :w],
                        lhsT=wqk_s[:, c, :],
                        rhs=hsT[:, si, c, :w],
                        start=(c == 0),
                        stop=(c == 7),
                    )
                # q on partitions 0:64 -> evacuate (+bq) straight into qT
                nc.vector.tensor_scalar_add(
                    qT[:, sb : sb + w], ps[0:64, :w], bq_s[:, 0:1]
                )
                # k on partitions 64:128 -> evacuate in place, bounce to
                # rows 0:64.  No k bias: it cancels in the softmax.
                nc.vector.tensor_copy(
                    kT_hi[64:128, sb : sb + w], ps[64:128, :w]
                )
                # SBUF->SBUF bounce rides the SWDGE: the sync ring is
                # busy with bulk h1 halves exactly when early bounces
                # must land, and SWDGE doesn't touch HBM bandwidth.
                nc.gpsimd.dma_start(
                    kT[:, sb : sb + w], kT_hi[64:128, sb : sb + w]
                )

            def copy_vS(ta, tb):
                # route chunks to the part-specific V tile on GpSimd
                # (idle here; keeps vector out of the AV dependency chain)
                if ta < NKC_P:
                    e = min(tb, NKC_P)
                    nc.gpsimd.tensor_copy(vS8[:, ta:e, 0:H], vN[:, ta:e, :])
                if tb > NKC_P:
                    b = max(ta, NKC_P)
                    nc.gpsimd.tensor_copy(
                        vS_bf[:, b - NKC_P : tb - NKC_P, 0:H], vN[:, b:tb, :]
                    )

            def emit_v_slice(sb):
                si = sb // 512
                w = min(512, SVP - sb)
                pvd = pp.tile([H, 512], f32, tag="ps", name="pvd")
                for c in range(8):
                    nc.tensor.matmul(
                        pvd[:, :w],
                        lhsT=wv_s[:, c, :],
                        rhs=hsT[:, si, c, :w],
                        start=(c == 0),
                        stop=(c == 7),
                    )
                nc.vector.tensor_copy(vT[:, sb : sb + w], pvd[:, :w])

            def emit_v_transpose(sa, sb_end):
                # one XBAR transpose covering [sa, sb_end) columns
                ta, tb = sa // P, sb_end // P
                nc.sync.dma_start_transpose(vN[:, ta:tb, :], vT[:, sa:sb_end])
                copy_vS(ta, tb)

            class QBlock:
                """One slice-aligned query block.  part 0 = pad
                (bidirectional keys 0..SP, fp8 DoubleRow AV), part 1 =
                valid (causal, bf16 AV).  Score pairs go into a 2-bank
                PSUM tile; exp is batched over the pair."""

                def __init__(self, part, q0r, w):
                    self.part = part
                    self.causal = part == 1
                    part_q0 = 0 if part == 0 else SP
                    self.kc_base = 0 if part == 0 else NKC_P
                    self.w = w
                    self.q0r = q0r
                    self.q0 = part_q0 + q0r
                    if self.causal:
                        self.kcs = list(range(0, (q0r + w - 1) // P + 1))
                    else:
                        self.kcs = list(range(NKC_P))
                    self.n_kc = len(self.kcs)
                    self.ki = 0
                    self.pend = []  # (kcr, wt_tile, j, y0, d)
                    self.ot = None
                    self.st = None
                    self.wt = None
                    self.slot = 0  # pair-slot within current st/wt tile
                    self.slots = 512 // self.w

                def _d(self, kcr):
                    return max(kcr * P - self.q0r, 0) if self.causal else 0

                def scores(self, kcrs, fillers=0):
                    if not kcrs:
                        return
                    if self.ot is None:
                        self.ot = acc.tile(
                            [H + 1, 512], f32, tag="acc", name="ot"
                        )
                    w, q0 = self.w, self.q0
                    i = 0
                    while i < len(kcrs):
                        pair = kcrs[i : i + 2]
                        i += len(pair)
                        if self.slot == 0:
                            self.st = spsum.tile(
                                [P, 2, 512], f32, tag="st", name="st"
                            )
                            if self.causal:
                                self.wt = wpool.tile(
                                    [P, 2, 512], bf, tag="wtb", name="wtb"
                                )
                            else:
                                self.wt = wpool.tile(
                                    [P, 2, 512], f8, tag="wt", name="wt"
                                )
                        st, wt, y0 = self.st, self.wt, self.slot * self.w
                        ds = [self._d(k) for k in pair]
                        for j, (kcr, d) in enumerate(zip(pair, ds)):
                            kc = self.kc_base + kcr
                            nc.tensor.matmul(
                                st[:, j, y0 + d : y0 + w],
                                lhsT=kT[:, kc * P : (kc + 1) * P],
                                rhs=qT[:, q0 + d : q0 + w],
                                start=True,
                                stop=True,
                                skip_group_check=True,
                            )
                        if fillers:
                            emit_filler(fillers)
                        if len(pair) == 2 and ds[0] == ds[1]:
                            nc.scalar.activation(
                                wt[:, 0:2, y0 + ds[0] : y0 + w],
                                st[:, 0:2, y0 + ds[0] : y0 + w],
                                Exp, bias=nbias[:, 0:1],
                            )
                        else:
                            for j, (kcr, d) in enumerate(zip(pair, ds)):
                                nc.scalar.activation(
                                    wt[:, j, y0 + d : y0 + w],
                                    st[:, j, y0 + d : y0 + w],
                                    Exp, bias=nbias[:, 0:1],
                                )
                        if self.causal:
                            for j, (kcr, d) in enumerate(zip(pair, ds)):
                                if kcr * P - self.q0r >= 0:
                                    # diagonal block: only the 128-wide
                                    # strip at d needs the intra-block
                                    # tril; later cols are all-keep
                                    de = min(d + P, w)
                                    nc.vector.tensor_mul(
                                        wt[:, j, y0 + d : y0 + de],
                                        wt[:, j, y0 + d : y0 + de],
                                        c01_s[:, 0 : de - d],
                                    )
                        for j, (kcr, d) in enumerate(zip(pair, ds)):
                            self.pend.append((kcr, wt, j, y0, d))
                        self.slot = (self.slot + 1) % self.slots

                def avs(self, n_chunks=None):
                    # consume pend'd exp'd chunks: fp8 DoubleRow pairs for
                    # pad, bf16 singles for causal
                    w, left = self.w, n_chunks
                    while self.pend and (left is None or left > 0):
                        if (
                            not self.causal
                            and len(self.pend) >= 2
                            and self.pend[0][1] is self.pend[1][1]
                            and self.pend[0][2] == 0
                            and self.pend[0][3] == self.pend[1][3]
                        ):
                            (kc0, wt, _, y0, _) = self.pend[0]
                            kc1 = self.pend[1][0]
                            assert kc1 == kc0 + 1
                            nc.tensor.matmul(
                                self.ot[:, :w],
                                lhsT=vS8[:, kc0 : kc0 + 2, 0 : H + 1],
                                rhs=wt[:, 0:2, y0 : y0 + w],
                                start=(self.ki == 0),
                                stop=(self.ki + 2 == self.n_kc),
                                perf_mode=DR,
                                skip_group_check=True,
                            )
                            self.ki += 2
                            del self.pend[:2]
                            if left is not None:
                                left -= 2
                        else:
                            kcr, wt, j, y0, d = self.pend[0]
                            vsrc = (
                                vS_bf[:, kcr, :]
                                if self.causal
                                else vS8[:, kcr, 0 : H + 1]
                            )
                            nc.tensor.matmul(
                                self.ot[:, d:w],
                                lhsT=vsrc,
                                rhs=wt[:, j, y0 + d : y0 + w],
                                start=(self.ki == 0),
                                stop=(self.ki + 1 == self.n_kc),
                                skip_group_check=True,
                            )
                            self.ki += 1
                            del self.pend[:1]
                            if left is not None:
                                left -= 1

                def finish(self, store_engine):
                    self.avs()
                    assert self.ki == self.n_kc
                    w = self.w
                    osb = opool.tile([H + 1, 512], f16)
                    nc.vector.tensor_copy(osb[:, :w], self.ot[:, :w])
                    store_engine.dma_start(
                        outT_d[:, self.q0 : self.q0 + w], osb[:, :w]
                    )

            # ------- master schedule -------
            # Pad blocks are 512-aligned within [0, SP); causal blocks
            # are aligned to the ABSOLUTE 512-grid within [SP, SVP) so
            # each causal block's queries live in exactly one hsT slice
            # and its scores can fire right after that slice's qk.
            slice_starts = list(range(0, SVP, 512))
            pad_blocks = [
                QBlock(0, r, min(512, SP - r)) for r in range(0, SP, 512)
            ]
            cuts = [SP] + [
                c for c in range(((SP // 512) + 1) * 512, SVP, 512)
            ] + [SVP]
            causal_blocks = [
                QBlock(1, a - SP, b - a) for a, b in zip(cuts, cuts[1:])
            ]
            # causal block -> index of the qk slice covering its queries
            c_by_slice = {}
            for blk in causal_blocks:
                c_by_slice.setdefault((SP + blk.q0r) // 512, []).append(blk)

            kc_s01 = min(NKC_P, 1024 // P)
            P0 = pad_blocks[0]
            P1 = pad_blocks[1] if len(pad_blocks) > 1 else None

            emit_qk_slice(slice_starts[0])
            emit_qk_slice(slice_starts[1])
            P0.scores(P0.kcs[:kc_s01])
            if P1 is not None:
                P1.scores(P1.kcs[:kc_s01])

            emit_v_slice(slice_starts[0])
            emit_v_slice(slice_starts[1])
            emit_v_transpose(0, min(1024, SVP))

            if NSL > 2:
                emit_qk_slice(slice_starts[2])
                # v2 fills the PE while kb2 (SWDGE) is in flight, and
                # its transpose covers pad chunk 8 for the pad AV8s
                emit_v_slice(slice_starts[2])
                emit_v_transpose(1024, min(1536, SVP))
            P0.scores(P0.kcs[kc_s01:])
            for blk in pad_blocks[2:]:
                blk.scores(blk.kcs)
            for blk in c_by_slice.get(2, []):
                blk.scores(blk.kcs, fillers=1)
            if NSL > 3:
                emit_qk_slice(slice_starts[3])
            P0.avs(2 * (kc_s01 // 2))
            if P1 is not None:
                P1.avs(2 * (kc_s01 // 2))
                P1.scores(P1.kcs[kc_s01:])
            for blk in c_by_slice.get(3, []):
                blk.scores(blk.kcs, fillers=1)

            for si in range(4, NSL):
                emit_qk_slice(slice_starts[si])
                for blk in c_by_slice.get(si, []):
                    blk.scores(blk.kcs, fillers=1)

            for sb in slice_starts[3:]:
                emit_v_slice(sb)
                emit_v_transpose(sb, min(sb + 512, SVP))

            # finishes in ot-pool rotation order (= scores order)
            fin_order = []
            seen = set()
            for blk in (
                pad_blocks[:2]
                + c_by_slice.get(2, [])
                + pad_blocks[2:]
                + [b for bs in sorted(c_by_slice) for b in c_by_slice[bs]]
            ):
                if id(blk) not in seen:
                    seen.add(id(blk))
                    fin_order.append(blk)
            for i, blk in enumerate(fin_order):
                last = i == len(fin_order) - 1
                blk.finish(nc.sync if last else nc.gpsimd)
    return nc


def _prepare(hidden_state, attention_masks, Wq, bq, Wk, bk, Wv, bv):
    """Host-side shard prep: sort each sequence into [pad | valid],
    pad both groups to shared multiples of 128, cast to bf16."""
    hs = np.asarray(hidden_state, dtype=np.float32)
    m = np.asarray(attention_masks)
    perms, nvs = [], []
    for b in range(B):
        mb = np.asarray(m[b]).astype(np.int64)
        perms.append(np.argsort(1 - mb, kind="stable"))
        nvs.append(int(mb.sum()))
    nps = [S - nv for nv in nvs]
    SV = max(128, -(-max(nvs) // P) * P)
    SPn = max(128, -(-max(nps) // P) * P)
    SVP = SV + SPn
    NKC_P, NKC_V = SPn // P, SV // P

    wqk = np.ascontiguousarray(
        np.concatenate(
            [np.asarray(Wq, np.float32) / np.sqrt(H), np.asarray(Wk, np.float32)],
            axis=1,
        ).reshape(8, P, P).transpose(1, 0, 2)
    ).astype(BF)  # [p, c, m]
    wv = np.ascontiguousarray(
        np.asarray(Wv, np.float32).reshape(8, P, H).transpose(1, 0, 2)
    ).astype(BF)  # [p, c, m]
    # packed constants: col 0 = bq/sqrt(H) (rows 0:64); cols 1:129 the
    # tril keep-mask c01[j, y] = 1.0 iff j <= y
    cst = np.zeros((P, 1 + P), np.float32)
    cst[:H, 0] = np.asarray(bq, np.float32) / np.sqrt(H)
    y = np.arange(P)
    cst[:, 1:] = np.arange(P)[:, None] <= y[None, :]
    cst = cst.astype(BF)

    in_maps = []
    for b in range(B):
        nv, npd = nvs[b], nps[b]
        NSL = (SVP + 511) // 512
        # pad-first layout: pad rows at cols [0, npd), valid rows at
        # [SPn, SPn + nv); slot-padding rows are zero and their keys are
        # killed through the V kill column
        hs_sorted = np.zeros((NSL * 512, E), np.float32)
        hs_sorted[:npd] = hs[b][perms[b][nv:]]
        hs_sorted[SPn : SPn + nv] = hs[b][perms[b][:nv]]
        hsT = np.ascontiguousarray(
            hs_sorted.reshape(NSL, 512, 8, P).transpose(3, 0, 2, 1)
        ).astype(BF)
        kill = np.ones((SVP,), np.float32)
        kill[npd:SPn] = 0.0
        kill[SPn + nv :] = 0.0
        # [P, NKC, 1] layout: kill[kc*128 + p] -> [p, kc, 0]
        killp = kill[:SPn].reshape(NKC_P, P).T[:, :, None]
        killv = kill[SPn:].reshape(NKC_V, P).T[:, :, None]
        in_maps.append(
            {
                "hsT": hsT,
                "wqk": wqk,
                "wv": wv,
                "cst": cst,
                "kill8": np.ascontiguousarray(killp).astype(F8),
                "killbf": np.ascontiguousarray(killv).astype(BF),
            }
        )
    return in_maps, perms, nvs, SV, SPn


def _run(inputs: dict, trace: bool = False):
    from concourse import bass_utils

    in_maps, perms, nvs, SV, SPn = _prepare(**inputs)
    key = (SV, SPn)
    if key not in _NC_CACHE:
        _NC_CACHE[key] = build_nc(SV, SPn)
    nc = _NC_CACHE[key]

    res = bass_utils.run_bass_kernel_spmd(
        nc, in_maps, core_ids=list(range(8)), trace=trace
    )

    bv = np.asarray(inputs["bv"], np.float32)
    out = np.empty((B, S, H), np.float32)
    for b in range(B):
        ot = np.asarray(res.results[b]["outT"], np.float32)  # [65, SVP]
        with np.errstate(divide="ignore", invalid="ignore", over="ignore"):
            dev = (ot[:H] / ot[H]).T  # normalized; slot-pad rows discarded
        nv = nvs[b]
        out[b][perms[b][nv:]] = dev[: S - nv]
        out[b][perms[b][:nv]] = dev[SPn : SPn + nv]
    out += bv  # v-projection bias commutes with the softmax average
    return out, res


def kernel(**inputs) -> np.ndarray:
    out, _ = _run(inputs, trace=False)
    return out


# revision 26
# speedup vs baseline: 1.1281x; 1.0028x over previous
"""Trainium2 Bass kernel for nn_AttentionHead_51178830299302.

Single attention head: B=8, S=2048, E=1024, H=64, fp32 I/O, decoder
(causal) masking plus a pad-pad coupling term (padded queries attend
bidirectionally to padded keys).

Strategy:
  * Data-parallel over batch: one batch element per NeuronCore (8 cores).
  * Host-side, each sequence is stably partitioned into [pad | valid]
    positions.  The masked softmax decomposes exactly into two
    independent attention problems: pad x pad (bidirectional, no mask)
    and valid x valid (plain causal), skipping ~60% of the S x S work.
  * The k-projection bias is dropped entirely: softmax_j((q+bq)·(k_j+bk))
    equals softmax_j((q+bq)·k_j) because the difference (q+bq)·bk is
    constant over j.  Scores contract only the 64 head dims (K=64), and
    slot-padded keys are killed through the appended ones-column of V
    (0 at killed rows), which removes them from both the numerator and
    the softmax denominator.
  * DMA is the backbone: the two HWDGE queues deliver only ~150 GB/s
    each here (vNC-shared HBM), so the ~6 MB input stream is what the
    schedule is built around.  Query blocks are slice-aligned and each
    block's scores are emitted immediately after the qk slice that
    covers its queries, keeping the ScalarE exp stream (the serial
    floor, ~17 us) dense from first data to last.  The DMA-completion
    semaphore pool is small and recycled in emission order, so the five
    fast constant loads are placed right before the bulk: they — not
    20 us-late bulk transfers — become the lane predecessors of the
    latency-critical k-bounces and V transposes.
  * Score pairs land in one 2-bank PSUM tile so exp() is batched per
    pair ([P, 2, w] straight from PSUM, bias -3 keeps the pad path in
    fp8 e4m3 range; e^-3 cancels in the host divide).  Pad softmax
    weights and V are fp8 e4m3, AV contracts 2 key-chunks per matmul
    via DoubleRow; the causal part stays bf16.
  * Emission interleaves projections with attention so the PE never
    idles long enough for the HAM activity monitor to drop the clock
    2.4->1.2 GHz, with zero-matmul fillers riding the exp-bound tail.
  * Output is fp16 (host divides in f32); most stores ride the GpSimd
    SWDGE, only the last block's store sits on the sync ring.

kernel(**inputs) takes the FULL unsharded fp32 inputs and returns the
FULL [8, 2048, 64] fp32 output.
"""

import numpy as np
import ml_dtypes

B, S, E, H = 8, 2048, 1024, 64
NEG = -100000.0
P = 128
BF = ml_dtypes.bfloat16
F8 = ml_dtypes.float8_e4m3
F16 = np.float16

_NC_CACHE: dict = {}


def _patch_tile_drain():
    """The stock TileContext exit hangs every global-clock wait on a single
    Drain instruction; this container's walrus caps sync waits at 1 per
    instruction.  Split the waits across single-wait nops, and drop the
    second (post-semclear) all-engine barrier — engines halt right after,
    and NEFF re-execution only starts once every engine has halted."""
    import concourse.tile as tile
    import concourse.mybir as mybir
    from bass_rust import ScopedClock

    if getattr(tile.TileContext, "_drain_waits_split", False):
        return

    def _drain_and_barrier(self, tick_clock, wait_clock):
        nc = self.nc
        carrier = nc.sync.nop(nofuse=True)
        wait_clock.add_sem_waits(
            carrier.ins, ScopedClock({None: tick_clock.global_clock})
        )
        si = carrier.ins.sync_info
        waits = list(si.on_wait) if si and si.on_wait else []
        if len(waits) > 1:
            si.on_wait = waits[:1]
            for w in waits[1:]:
                n = nc.sync.nop(nofuse=True)
                nsi = n.ins.sync_info
                if nsi is None:
                    n.ins.sync_info = mybir.SyncInfo(on_wait=[w], on_update=[])
                else:
                    nsi.on_wait = [w]
        nc.sync.drain()
        nc.all_engine_barrier(sem_only=True)
        popped = nc._tile_sem_poison_stack.pop()
        assert popped is self._sem_poison
        nc.clear_and_free_semaphores(list(self.sems.allocated().values()))

    tile.TileContext._drain_and_barrier = _drain_and_barrier
    tile.TileContext._drain_waits_split = True


def _patch_sync_wait_split():
    """This container's walrus codegen rejects instructions carrying more
    than one sync wait.  Post-process the serialized BIR: hoist excess
    waits onto injected NoOps on the same engine, just before the
    instruction (the sequencer executes them in order, so semantics are
    preserved)."""
    import json
    import concourse.bass as bass

    if getattr(bass.Bass, "_sync_wait_split", False):
        return
    orig = bass.Bass.to_json_bytes

    def to_json_bytes(self) -> bytes:
        j = json.loads(orig(self))
        ctr = [0]

        def fix_block(blk):
            insts = blk.get("instructions")
            if not isinstance(insts, list):
                return
            out = []
            for inst in insts:
                si = inst.get("sync_info")
                ow = (si or {}).get("on_wait") or []
                if len(ow) > 1:
                    si["on_wait"] = ow[-1:]
                    for w in ow[:-1]:
                        ctr[0] += 1
                        out.append(
                            {
                                "debug": inst.get("debug", 0),
                                "engine": inst["engine"],
                                "ins": [],
                                "name": f"I-wsplit-{ctr[0]}",
                                "opcode": "NoOp",
                                "outs": [],
                                "sync_info": {"on_wait": [w], "on_update": []},
                            }
                        )
                out.append(inst)
            blk["instructions"] = out

        def rec(o):
            if isinstance(o, dict):
                if "instructions" in o:
                    fix_block(o)
                for v in o.values():
                    rec(v)
            elif isinstance(o, list):
                for v in o:
                    rec(v)

        rec(j)
        return json.dumps(j).encode()

    bass.Bass.to_json_bytes = to_json_bytes
    bass.Bass._sync_wait_split = True


def build_nc(SV: int, SP: int):
    """Build the SPMD per-core Bass program.

    Per-core DRAM tensors:
      hsT    [P, NSL, 8, 512] bf16  sorted hidden state, transposed
      wqk    [P, 8, 128]      bf16  [Wq/sqrt(H) | Wk]
      wv     [P, 8, H]        bf16
      cst    [P, 129]         bf16  col 0 bq/sqrt(H); cols 1:129 the
                                    tril keep-mask c01[j, y] = (j <= y)
      kill8  [P, NKC_P, 1]    f8    1 at real pad keys, 0 at slot-pads
      killbf [P, NKC_V, 1]    bf16  1 at real valid keys, 0 at slot-pads
      outT   [65, SVP]        f16   rows 0..63 unnormalized output^T,
                                    row 64 softmax denominators
    """
    import concourse.bass as bass
    import concourse.mybir as mybir
    import concourse.tile as tile
    from contextlib import ExitStack

    _patch_tile_drain()
    _patch_sync_wait_split()
    bf, f32, f16 = mybir.dt.bfloat16, mybir.dt.float32, mybir.dt.float16
    f8 = mybir.dt.float8e4
    DR = mybir.MatmulPerfMode.DoubleRow
    Exp = mybir.ActivationFunctionType.Exp

    SVP = SV + SP
    NKC_V, NKC_P = SV // P, SP // P
    NT = SVP // P
    NSL = (SVP + 511) // 512

    nc = bass.Bass("TRN2", target_bir_lowering=False, debug=False)
    hsT_d = nc.dram_tensor("hsT", [P, NSL, 8, 512], bf, kind="ExternalInput").ap()
    wqk_d = nc.dram_tensor("wqk", [P, 8, P], bf, kind="ExternalInput").ap()
    wv_d = nc.dram_tensor("wv", [P, 8, H], bf, kind="ExternalInput").ap()
    cst_d = nc.dram_tensor("cst", [P, 1 + P], bf, kind="ExternalInput").ap()
    kill8_d = nc.dram_tensor("kill8", [P, NKC_P, 1], f8, kind="ExternalInput").ap()
    killbf_d = nc.dram_tensor(
        "killbf", [P, NKC_V, 1], bf, kind="ExternalInput"
    ).ap()
    outT_d = nc.dram_tensor("outT", [H + 1, SVP], f16, kind="ExternalOutput").ap()

    with tile.TileContext(nc) as tc, ExitStack() as ctx:
        singles = ctx.enter_context(tc.tile_pool(name="singles", bufs=1))

        wz = singles.tile([P, 512], bf)
        nc.gpsimd.memset(wz[:], 0.0)

        wqk_s = singles.tile([P, 8, P], bf)
        wv_s = singles.tile([P, 8, H], bf)
        # packed constants: col 0 = bq/sqrt(H) (rows 0:64, bf16), cols
        # 1:129 = the c01 tril keep-mask
        cst_s = singles.tile([P, 1 + P], bf)
        c01_s = cst_s[:, 1 : 1 + P]
        bq_s = singles.tile([H, 1], f32)
        exp_warm = singles.tile([1, 1], bf)
        # exp runs as exp(s - 3) so the pad path stays inside fp8 e4m3
        # range; the factor e^-3 cancels in the host num/den divide.
        nbias = singles.tile([P, 1], f32)
        nc.gpsimd.memset(nbias[:], -3.0)

        # q/k head rows (no augmented rows: slot-pad keys are killed via
        # the V kill column, keeping score K=64).  k projects to PSUM
        # rows 64:128 and evacuates in place to kT_hi; a DMA bounce
        # brings it down to kT rows 0:64 where the score matmuls (whose
        # operands must share a partition range) read it.
        qT = singles.tile([H, SVP], bf)
        kT = singles.tile([H, SVP], bf)
        kT_hi = singles.tile([P, SVP], bf)

        # V in natural [key-part, head] layout; column H holds the kill
        # 0/1 flags and rides the AV matmul as the softmax denominator
        # row.  fp8 for pad chunks (DoubleRow AV; free dim padded 65->80
        # for the 16B DoubleRow chunk stride), bf16 for valid chunks.
        vS8 = singles.tile([P, NKC_P, 80], f8)
        vS_bf = singles.tile([P, NKC_V, H + 1], bf)
        vT = singles.tile([H, SVP], bf)
        # XBAR transpose needs a contiguous destination; stage here, then
        # strided-copy into vS (which carries the kill column).
        vN = singles.tile([P, NT, H], bf)
        # kill flags come in contiguous (tiny strided DMAs are glacial
        # on the SWDGE) and are copied into the vS kill columns on-chip
        k8st = singles.tile([P, NKC_P, 1], f8)
        kbfst = singles.tile([P, NKC_V, 1], bf)

        hsT = singles.tile([P, NSL, 8, 512], bf)

        # ------- DMA plan -------
        # Emission order doubles as the DMA-sem lane order; constants sit
        # right before the bulk so later latency-critical transfers have
        # fast lane predecessors.  Scalar(Q10) carries the h0 halves plus
        # the whole tail slice and then goes quiet for exp; sync(Q1)
        # carries wqk + early h1 halves, then serves bounces/transposes
        # just-in-time (s2h1/s3h1 are emitted mid-schedule so they don't
        # block those in the ring FIFO).
        def load_half(si, h, eng):
            w2 = min(512, SVP - si * 512)
            eng.dma_start(
                hsT[:, si, 4 * h : 4 * h + 4, :w2],
                hsT_d[:, si, 4 * h : 4 * h + 4, :w2],
            )

        def load_full(si, eng):
            w2 = min(512, SVP - si * 512)
            eng.dma_start(hsT[:, si, :, :w2], hsT_d[:, si, :, :w2])

        # The DMA-completion sem pool holds ~13 entries, recycled in
        # scheduled order; a trigger whose recycled sem isn't free yet
        # blocks the whole engine queue behind it (on scalar that queue
        # carries the exp stream).  Keep the pre-compute transfer count
        # AT the pool size so no pre-compute trigger ever waits, and all
        # mid-schedule latency-critical transfers recycle the sems of
        # fast early transfers.
        nc.scalar.dma_start(wqk_s[:], wqk_d)
        load_half(0, 1, nc.sync)
        load_half(0, 0, nc.scalar)
        load_half(1, 0, nc.scalar)
        load_half(1, 1, nc.sync)
        load_half(2, 0, nc.scalar)
        load_half(2, 1, nc.sync)
        for si in range(3, NSL):
            load_full(si, nc.scalar)
        nc.gpsimd.dma_start(wv_s[:], wv_d)
        nc.gpsimd.dma_start(cst_s[:], cst_d)
        nc.gpsimd.dma_start(k8st[:], kill8_d)
        nc.gpsimd.dma_start(kbfst[:], killbf_d)
        nc.gpsimd.tensor_copy(vS8[:, :, H : H + 1], k8st[:])
        nc.gpsimd.tensor_copy(vS_bf[:, :, H : H + 1], kbfst[:])
        nc.gpsimd.tensor_copy(bq_s[:], cst_s[0:H, 0:1])  # bf16 -> f32

        with tc.tile_pool(name="pp", bufs=2, space="PSUM") as pp, \
             tc.tile_pool(name="acc", bufs=2, space="PSUM") as acc, \
             tc.tile_pool(name="spsum", bufs=2, space="PSUM") as spsum, \
             tc.tile_pool(name="wpool", bufs=8) as wpool, \
             tc.tile_pool(name="opool", bufs=2) as opool:

            # dummy exp pulls the ~2.7us ACT_TABLE_LOAD out of the
            # attention pipeline (right after scalar's DMA triggers).
            nc.scalar.activation(exp_warm[:], wz[0:1, 0:1], Exp)
            # PE warm-up: fills the time until hsT slice 0 lands and
            # ramps the HAM p-state (1.2 -> 2.4 GHz after ~3.4us busy).
            warm_ps = pp.tile([P, 512], f32, tag="ps", name="warm_ps")
            for _ in range(14):
                nc.tensor.matmul(
                    warm_ps[:, 0:256], lhsT=wz[:, 0:P], rhs=wz[:, 0:256],
                    start=True, stop=True,
                )

            def emit_filler(n):
                # keep-warm zero matmuls: execute during exp waits in the
                # attention tail so the HAM activity window stays busy.
                fps = pp.tile([P, 512], f32, tag="ps", name="fps")
                for _ in range(n):
                    nc.tensor.matmul(
                        fps[:, 0:256], lhsT=wz[:, 0:P], rhs=wz[:, 0:256],
                        start=True, stop=True, skip_group_check=True,
                    )

            def emit_qk_slice(sb):
                si = sb // 512
                w = min(512, SVP - sb)
                ps = pp.tile([P, 512], f32, tag="ps", name="ps")
                for c in range(8):
                    nc.tensor.matmul(
                        ps[:, :w],
                        lhsT=wqk_s[:, c, :],
                        rhs=hsT[:, si, c, :w],
                        start=(c == 0),
                        stop=(c == 7),
                    )
                # q on partitions 0:64 -> evacuate (+bq) straight into qT
                nc.vector.tensor_scalar_add(
                    qT[:, sb : sb + w], ps[0:64, :w], bq_s[:, 0:1]
                )
                # k on partitions 64:128 -> evacuate in place, bounce to
                # rows 0:64.  No k bias: it cancels in the softmax.
                nc.vector.tensor_copy(
                    kT_hi[64:128, sb : sb + w], ps[64:128, :w]
                )
                # the bounce rides the sync ring: its trigger fires
                # after the k evacuation (~20us+), by which time the
                # ring has drained its bulk h1 halves, and the HWDGE
                # round trip is ~2us quicker than the SWDGE's.
                nc.sync.dma_start(
                    kT[:, sb : sb + w], kT_hi[64:128, sb : sb + w]
                )

            def copy_vS(ta, tb):
                # route chunks to the part-specific V tile on GpSimd
                # (idle here; keeps vector out of the AV dependency chain)
                if ta < NKC_P:
                    e = min(tb, NKC_P)
                    nc.gpsimd.tensor_copy(vS8[:, ta:e, 0:H], vN[:, ta:e, :])
                if tb > NKC_P:
                    b = max(ta, NKC_P)
                    nc.gpsimd.tensor_copy(
                        vS_bf[:, b - NKC_P : tb - NKC_P, 0:H], vN[:, b:tb, :]
                    )

            def emit_v_slice(sb):
                si = sb // 512
                w = min(512, SVP - sb)
                pvd = pp.tile([H, 512], f32, tag="ps", name="pvd")
                for c in range(8):
                    nc.tensor.matmul(
                        pvd[:, :w],
                        lhsT=wv_s[:, c, :],
                        rhs=hsT[:, si, c, :w],
                        start=(c == 0),
                        stop=(c == 7),
                    )
                nc.vector.tensor_copy(vT[:, sb : sb + w], pvd[:, :w])

            def emit_v_transpose(sa, sb_end):
                # one XBAR transpose covering [sa, sb_end) columns
                ta, tb = sa // P, sb_end // P
                nc.sync.dma_start_transpose(vN[:, ta:tb, :], vT[:, sa:sb_end])
                copy_vS(ta, tb)

            class QBlock:
                """One slice-aligned query block.  part 0 = pad
                (bidirectional keys 0..SP, fp8 DoubleRow AV), part 1 =
                valid (causal, bf16 AV).  Score pairs go into a 2-bank
                PSUM tile; exp is batched over the pair."""

                def __init__(self, part, q0r, w):
                    self.part = part
                    self.causal = part == 1
                    part_q0 = 0 if part == 0 else SP
                    self.kc_base = 0 if part == 0 else NKC_P
                    self.w = w
                    self.q0r = q0r
                    self.q0 = part_q0 + q0r
                    if self.causal:
                        self.kcs = list(range(0, (q0r + w - 1) // P + 1))
                    else:
                        self.kcs = list(range(NKC_P))
                    self.n_kc = len(self.kcs)
                    self.ki = 0
                    self.pend = []  # (kcr, wt_tile, j, y0, d)
                    self.ot = None
                    self.st = None
                    self.wt = None
                    self.slot = 0  # pair-slot within current st/wt tile
                    self.slots = 512 // self.w

                def _d(self, kcr):
                    return max(kcr * P - self.q0r, 0) if self.causal else 0

                def scores(self, kcrs, fillers=0):
                    if not kcrs:
                        return
                    if self.ot is None:
                        self.ot = acc.tile(
                            [H + 1, 512], f32, tag="acc", name="ot"
                        )
                    w, q0 = self.w, self.q0
                    i = 0
                    while i < len(kcrs):
                        pair = kcrs[i : i + 2]
                        i += len(pair)
                        if self.slot == 0:
                            self.st = spsum.tile(
                                [P, 2, 512], f32, tag="st", name="st"
                            )
                            if self.causal:
                                self.wt = wpool.tile(
                                    [P, 2, 512], bf, tag="wtb", name="wtb"
                                )
                            else:
                                self.wt = wpool.tile(
                                    [P, 2, 512], f8, tag="wt", name="wt"
                                )
                        st, wt, y0 = self.st, self.wt, self.slot * self.w
                        ds = [self._d(k) for k in pair]
                        for j, (kcr, d) in enumerate(zip(pair, ds)):
                            kc = self.kc_base + kcr
                            nc.tensor.matmul(
                                st[:, j, y0 + d : y0 + w],
                                lhsT=kT[:, kc * P : (kc + 1) * P],
                                rhs=qT[:, q0 + d : q0 + w],
                                start=True,
                                stop=True,
                                skip_group_check=True,
                            )
                        if fillers:
                            emit_filler(fillers)
                        if len(pair) == 2 and ds[0] == ds[1]:
                            nc.scalar.activation(
                                wt[:, 0:2, y0 + ds[0] : y0 + w],
                                st[:, 0:2, y0 + ds[0] : y0 + w],
                                Exp, bias=nbias[:, 0:1],
                            )
                        else:
                            for j, (kcr, d) in enumerate(zip(pair, ds)):
                                nc.scalar.activation(
                                    wt[:, j, y0 + d : y0 + w],
                                    st[:, j, y0 + d : y0 + w],
                                    Exp, bias=nbias[:, 0:1],
                                )
                        if self.causal:
                            for j, (kcr, d) in enumerate(zip(pair, ds)):
                                if kcr * P - self.q0r >= 0:
                                    # diagonal block: only the 128-wide
                                    # strip at d needs the intra-block
                                    # tril; later cols are all-keep
                                    de = min(d + P, w)
                                    nc.vector.tensor_mul(
                                        wt[:, j, y0 + d : y0 + de],
                                        wt[:, j, y0 + d : y0 + de],
                                        c01_s[:, 0 : de - d],
                                    )
                        for j, (kcr, d) in enumerate(zip(pair, ds)):
                            self.pend.append((kcr, wt, j, y0, d))
                        self.slot = (self.slot + 1) % self.slots

                def avs(self, n_chunks=None):
                    # consume pend'd exp'd chunks: fp8 DoubleRow pairs for
                    # pad, bf16 singles for causal
                    w, left = self.w, n_chunks
                    while self.pend and (left is None or left > 0):
                        if (
                            not self.causal
                            and len(self.pend) >= 2
                            and self.pend[0][1] is self.pend[1][1]
                            and self.pend[0][2] == 0
                            and self.pend[0][3] == self.pend[1][3]
                        ):
                            (kc0, wt, _, y0, _) = self.pend[0]
                            kc1 = self.pend[1][0]
                            assert kc1 == kc0 + 1
                            nc.tensor.matmul(
                                self.ot[:, :w],
                                lhsT=vS8[:, kc0 : kc0 + 2, 0 : H + 1],
                                rhs=wt[:, 0:2, y0 : y0 + w],
                                start=(self.ki == 0),
                                stop=(self.ki + 2 == self.n_kc),
                                perf_mode=DR,
                                skip_group_check=True,
                            )
                            self.ki += 2
                            del self.pend[:2]
                            if left is not None:
                                left -= 2
                        else:
                            kcr, wt, j, y0, d = self.pend[0]
                            vsrc = (
                                vS_bf[:, kcr, :]
                                if self.causal
                                else vS8[:, kcr, 0 : H + 1]
                            )
                            nc.tensor.matmul(
                                self.ot[:, d:w],
                                lhsT=vsrc,
                                rhs=wt[:, j, y0 + d : y0 + w],
                                start=(self.ki == 0),
                                stop=(self.ki + 1 == self.n_kc),
                                skip_group_check=True,
                            )
                            self.ki += 1
                            del self.pend[:1]
                            if left is not None:
                                left -= 1

                def finish(self, store_engine):
                    self.avs()
                    assert self.ki == self.n_kc
                    w = self.w
                    osb = opool.tile([H + 1, 512], f16)
                    nc.vector.tensor_copy(osb[:, :w], self.ot[:, :w])
                    store_engine.dma_start(
                        outT_d[:, self.q0 : self.q0 + w], osb[:, :w]
                    )

            # ------- master schedule -------
            # Pad blocks are 512-aligned within [0, SP); causal blocks
            # are aligned to the ABSOLUTE 512-grid within [SP, SVP) so
            # each causal block's queries live in exactly one hsT slice
            # and its scores can fire right after that slice's qk.
            slice_starts = list(range(0, SVP, 512))
            pad_blocks = [
                QBlock(0, r, min(512, SP - r)) for r in range(0, SP, 512)
            ]
            cuts = [SP] + [
                c for c in range(((SP // 512) + 1) * 512, SVP, 512)
            ] + [SVP]
            causal_blocks = [
                QBlock(1, a - SP, b - a) for a, b in zip(cuts, cuts[1:])
            ]
            # causal block -> index of the qk slice covering its queries
            c_by_slice = {}
            for blk in causal_blocks:
                c_by_slice.setdefault((SP + blk.q0r) // 512, []).append(blk)

            kc_s01 = min(NKC_P, 1024 // P)
            P0 = pad_blocks[0]
            P1 = pad_blocks[1] if len(pad_blocks) > 1 else None

            emit_qk_slice(slice_starts[0])
            emit_qk_slice(slice_starts[1])
            P0.scores(P0.kcs[:kc_s01])
            if P1 is not None:
                P1.scores(P1.kcs[:kc_s01])

            emit_v_slice(slice_starts[0])
            emit_v_slice(slice_starts[1])
            emit_v_transpose(0, min(1024, SVP))

            if NSL > 2:
                emit_qk_slice(slice_starts[2])
                # v2 fills the PE while kb2 (SWDGE) is in flight, and
                # its transpose covers pad chunk 8 for the pad AV8s
                emit_v_slice(slice_starts[2])
                emit_v_transpose(1024, min(1536, SVP))
            P0.scores(P0.kcs[kc_s01:])
            for blk in pad_blocks[2:]:
                blk.scores(blk.kcs)
            for blk in c_by_slice.get(2, []):
                blk.scores(blk.kcs, fillers=1)
            if NSL > 3:
                emit_qk_slice(slice_starts[3])
            P0.avs(2 * (kc_s01 // 2))
            if P1 is not None:
                P1.avs(2 * (kc_s01 // 2))
                P1.scores(P1.kcs[kc_s01:])
            for blk in c_by_slice.get(3, []):
                blk.scores(blk.kcs, fillers=1)

            for si in range(4, NSL):
                emit_qk_slice(slice_starts[si])
                for blk in c_by_slice.get(si, []):
                    blk.scores(blk.kcs, fillers=1)

            for sb in slice_starts[3:]:
                emit_v_slice(sb)
                emit_v_transpose(sb, min(sb + 512, SVP))

            # finishes in ot-pool rotation order (= scores order)
            fin_order = []
            seen = set()
            for blk in (
                pad_blocks[:2]
                + c_by_slice.get(2, [])
                + pad_blocks[2:]
                + [b for bs in sorted(c_by_slice) for b in c_by_slice[bs]]
            ):
                if id(blk) not in seen:
                    seen.add(id(blk))
                    fin_order.append(blk)
            for i, blk in enumerate(fin_order):
                last = i == len(fin_order) - 1
                blk.finish(nc.sync if last else nc.gpsimd)
    return nc


def _prepare(hidden_state, attention_masks, Wq, bq, Wk, bk, Wv, bv):
    """Host-side shard prep: sort each sequence into [pad | valid],
    pad both groups to shared multiples of 128, cast to bf16."""
    hs = np.asarray(hidden_state, dtype=np.float32)
    m = np.asarray(attention_masks)
    perms, nvs = [], []
    for b in range(B):
        mb = np.asarray(m[b]).astype(np.int64)
        perms.append(np.argsort(1 - mb, kind="stable"))
        nvs.append(int(mb.sum()))
    nps = [S - nv for nv in nvs]
    SV = max(128, -(-max(nvs) // P) * P)
    SPn = max(128, -(-max(nps) // P) * P)
    SVP = SV + SPn
    NKC_P, NKC_V = SPn // P, SV // P

    wqk = np.ascontiguousarray(
        np.concatenate(
            [np.asarray(Wq, np.float32) / np.sqrt(H), np.asarray(Wk, np.float32)],
            axis=1,
        ).reshape(8, P, P).transpose(1, 0, 2)
    ).astype(BF)  # [p, c, m]
    wv = np.ascontiguousarray(
        np.asarray(Wv, np.float32).reshape(8, P, H).transpose(1, 0, 2)
    ).astype(BF)  # [p, c, m]
    # packed constants: col 0 = bq/sqrt(H) (rows 0:64); cols 1:129 the
    # tril keep-mask c01[j, y] = 1.0 iff j <= y
    cst = np.zeros((P, 1 + P), np.float32)
    cst[:H, 0] = np.asarray(bq, np.float32) / np.sqrt(H)
    y = np.arange(P)
    cst[:, 1:] = np.arange(P)[:, None] <= y[None, :]
    cst = cst.astype(BF)

    in_maps = []
    for b in range(B):
        nv, npd = nvs[b], nps[b]
        NSL = (SVP + 511) // 512
        # pad-first layout: pad rows at cols [0, npd), valid rows at
        # [SPn, SPn + nv); slot-padding rows are zero and their keys are
        # killed through the V kill column
        hs_sorted = np.zeros((NSL * 512, E), np.float32)
        hs_sorted[:npd] = hs[b][perms[b][nv:]]
        hs_sorted[SPn : SPn + nv] = hs[b][perms[b][:nv]]
        hsT = np.ascontiguousarray(
            hs_sorted.reshape(NSL, 512, 8, P).transpose(3, 0, 2, 1)
        ).astype(BF)
        kill = np.ones((SVP,), np.float32)
        kill[npd:SPn] = 0.0
        kill[SPn + nv :] = 0.0
        # [P, NKC, 1] layout: kill[kc*128 + p] -> [p, kc, 0]
        killp = kill[:SPn].reshape(NKC_P, P).T[:, :, None]
        killv = kill[SPn:].reshape(NKC_V, P).T[:, :, None]
        in_maps.append(
            {
                "hsT": hsT,
                "wqk": wqk,
                "wv": wv,
                "cst": cst,
                "kill8": np.ascontiguousarray(killp).astype(F8),
                "killbf": np.ascontiguousarray(killv).astype(BF),
            }
        )
    return in_maps, perms, nvs, SV, SPn


def _run(inputs: dict, trace: bool = False):
    from concourse import bass_utils

    in_maps, perms, nvs, SV, SPn = _prepare(**inputs)
    key = (SV, SPn)
    if key not in _NC_CACHE:
        _NC_CACHE[key] = build_nc(SV, SPn)
    nc = _NC_CACHE[key]

    res = bass_utils.run_bass_kernel_spmd(
        nc, in_maps, core_ids=list(range(8)), trace=trace
    )

    bv = np.asarray(inputs["bv"], np.float32)
    out = np.empty((B, S, H), np.float32)
    for b in range(B):
        ot = np.asarray(res.results[b]["outT"], np.float32)  # [65, SVP]
        with np.errstate(divide="ignore", invalid="ignore", over="ignore"):
            dev = (ot[:H] / ot[H]).T  # normalized; slot-pad rows discarded
        nv = nvs[b]
        out[b][perms[b][nv:]] = dev[: S - nv]
        out[b][perms[b][:nv]] = dev[SPn : SPn + nv]
    out += bv  # v-projection bias commutes with the softmax average
    return out, res


def kernel(**inputs) -> np.ndarray:
    out, _ = _run(inputs, trace=False)
    return out
